# revision 15
# baseline (speedup 1.0000x reference)
"""Causal multi-head attention (B=4, T=2048, C=1024, H=16) on 8 TRN2 NeuronCores.

Sharding: core c handles batch b=c//2 and head-half r=c%2 (8 of 16 heads).
Every core runs an IDENTICAL graph (full causal attention for its 8 heads over
all T tokens) -> pure SPMD, no collectives. The output projection is
row-parallel over the head-halves; the host sums the two partial z's per batch
(the unshard step) and adds the bias-fold vector Wp@bv.

Device layout choices:
  - everything enters as bf16 (host pre-casts); matmuls accumulate fp32 in PSUM
  - all DRAM inputs are PRE-ARRANGED on the host to match their SBUF
    destination exactly (partition-major slabs) -> every load DMA is one
    contiguous descriptor per partition, so inputs land at line rate
  - a short chain of dummy matmuls on a memset tile runs during the input
    DMA shadow, warming the PE HAM clock gate before the first real matmul
  - qT/kT stored [d, t] with head pairs stacked 64+64 on partitions ->
    K=64 row-tiled matmul pairs use both halves of the PE array concurrently
  - scores computed transposed S^T=[k, q]; exp on ScalarE (scale=1/8 fused);
    causal handled by N-trimming each matmul + one 128x128 triangle mask mul
  - softmax denominator l = sum_k exp computed for free by an all-ones column
    appended to v (65-wide weights, fp32 PSUM accumulation); y^T = v_aug^T@P^T
  - 1/l via ScalarE Ln then Exp(scale=-1) directly on the [1, 2, 512] l-rows
    (both functions live in one ACT table set), then one gpsimd
    partition_broadcast; normalize is a bf16 DVE multiply
"""

import os
from contextlib import ExitStack

import numpy as np
import ml_dtypes

import concourse.tile as tile
from concourse import bacc, mybir


def _ensure_axon_hooks():
    """bass_utils' axon trace path does a hard import of antenv.axon_hooks,
    which this image's antenv lacks. Provide the module (with a real NTFF
    hook when the axon .so supports it) so trace=True / BASS_TRACE=1 works;
    harmless when tracing is off."""
    try:
        import antenv.axon_hooks  # noqa: F401
        return
    except ImportError:
        pass
    import sys
    import types
    try:
        import antenv
    except ImportError:
        return
    m = types.ModuleType("antenv.axon_hooks")
    m._hook = None

    def set_axon_ntff_profile_hook(h):
        m._hook = h

    def get_axon_ntff_profile_hook():
        return m._hook

    m.set_axon_ntff_profile_hook = set_axon_ntff_profile_hook
    m.get_axon_ntff_profile_hook = get_axon_ntff_profile_hook
    sys.modules["antenv.axon_hooks"] = m
    antenv.axon_hooks = m


_ensure_axon_hooks()

from concourse.bass_utils import run_bass_kernel_spmd  # noqa: E402

BF = ml_dtypes.bfloat16
B, T, C, H, HD = 4, 2048, 1024, 16, 64
NCORES = 8
DH = C // 2        # 512 d-dims per core (8 heads)
NPAIR = 4          # head pairs per core
NQB = T // 512     # 4 query blocks of 512
NKB = T // 128     # 16 key/token blocks of 128
NCH = C // 128     # 8 contraction chunks
f32 = mybir.dt.float32
bf16 = mybir.dt.bfloat16

_CACHED_NC = None
LAST_RESULTS = None  # BassKernelResults of the most recent run


def _build_nc():
    nc = bacc.Bacc("TRN2", target_bir_lowering=False, debug=False,
                   num_devices=NCORES)
    AF = mybir.ActivationFunctionType

    # DRAM inputs pre-arranged host-side: leading dim = SBUF partition, and
    # each partition's slab is contiguous -> 1 DMA descriptor per partition.
    xT4D = nc.dram_tensor("xT4", [4, 128, NCH, 512], bf16,
                          kind="ExternalInput").ap()
    wqD = nc.dram_tensor("wq", [128, NPAIR, NCH, 128], bf16,
                         kind="ExternalInput").ap()
    wkD = nc.dram_tensor("wk", [128, NPAIR, NCH, 128], bf16,
                         kind="ExternalInput").ap()
    wvD = nc.dram_tensor("wv", [128, NCH, DH], bf16,
                         kind="ExternalInput").ap()
    wpD = nc.dram_tensor("wp", [128, NPAIR, C], bf16,
                         kind="ExternalInput").ap()
    bqkD = nc.dram_tensor("bqk", [128, 2, NPAIR, 1], f32,
                          kind="ExternalInput").ap()
    triD = nc.dram_tensor("tri", [128, 2, 128], bf16,
                          kind="ExternalInput").ap()
    zD = nc.dram_tensor("z", [T, C], mybir.dt.float16,
                        kind="ExternalOutput").ap()

    with tile.TileContext(nc) as tc, ExitStack() as ctx:
        const = ctx.enter_context(tc.tile_pool(name="const", bufs=1))
        qkp = ctx.enter_context(tc.tile_pool(name="qk", bufs=1))
        vp = ctx.enter_context(tc.tile_pool(name="vp", bufs=1))
        ynp = ctx.enter_context(tc.tile_pool(name="yn", bufs=1))
        ptp = ctx.enter_context(tc.tile_pool(name="pt", bufs=7))
        smallp = ctx.enter_context(tc.tile_pool(name="small", bufs=2))
        bcp = ctx.enter_context(tc.tile_pool(name="bc", bufs=2))
        zstp = ctx.enter_context(tc.tile_pool(name="zst", bufs=3))
        yevp = ctx.enter_context(tc.tile_pool(name="yev", bufs=6))
        stgp = ctx.enter_context(tc.tile_pool(name="stg", bufs=2))
        ps = ctx.enter_context(tc.tile_pool(name="ps", bufs=2, space="PSUM"))
        ps2 = ctx.enter_context(tc.tile_pool(name="ps2", bufs=2, space="PSUM"))
        yps = ctx.enter_context(tc.tile_pool(name="yps", bufs=2, space="PSUM"))

        # ---- input DMAs first so every queue starts pulling immediately
        tri2 = const.tile([128, 2, 128], bf16, tag="tri2")
        nc.gpsimd.dma_start(out=tri2[:, :, :], in_=triD)
        bqk = const.tile([128, 2, NPAIR, 1], f32, tag="bqk")
        nc.gpsimd.dma_start(out=bqk[:, :, :, :], in_=bqkD)
        bq_sb = [bqk[:, 0, hp, :] for hp in range(NPAIR)]
        bk_sb = [bqk[:, 1, hp, :] for hp in range(NPAIR)]

        xT4 = const.tile([128, 4, NCH, 512], bf16, tag="xT4", name="xT4")
        wqt = const.tile([128, NPAIR, NCH, 128], bf16, tag="wqt", name="wqt")
        wkt = const.tile([128, NPAIR, NCH, 128], bf16, tag="wkt", name="wkt")
        wvt = const.tile([128, NCH, DH], bf16, tag="wvt", name="wvt")
        wpt = const.tile([128, NPAIR, C], bf16, tag="wpt", name="wpt")

        # wave order tuned to the per-queue DMA rates (~100-140GB/s on the
        # sync/scalar HW queues, ~45GB/s on gpsimd's SWDGE queue) and to when
        # each consumer fires: pair-0 j=0 projections first, v weights early
        # on scalar so the v-projection fills the pre-attention bubble, the
        # remaining token quarters just-in-time behind them
        nc.sync.dma_start(out=wqt[:, 0:1, :, :], in_=wqD[:, 0:1, :, :])
        nc.scalar.dma_start(out=wkt[:, 0:1, :, :], in_=wkD[:, 0:1, :, :])
        for c2 in range(4):
            nc.sync.dma_start(out=xT4[:, 0, 2 * c2:2 * c2 + 2, :],
                              in_=xT4D[0, :, 2 * c2:2 * c2 + 2, :])
        nc.scalar.dma_start(out=wvt[:, :, :], in_=wvD)
        for hh in range(1, NPAIR):
            nc.sync.dma_start(out=wqt[:, hh:hh + 1, :, :],
                              in_=wqD[:, hh:hh + 1, :, :])
            nc.scalar.dma_start(out=wkt[:, hh:hh + 1, :, :],
                                in_=wkD[:, hh:hh + 1, :, :])
        nc.sync.dma_start(out=xT4[:, 1, :, :], in_=xT4D[1])
        nc.scalar.dma_start(out=xT4[:, 2, :, :], in_=xT4D[2])
        nc.sync.dma_start(out=xT4[:, 3, :, :], in_=xT4D[3])
        nc.gpsimd.dma_start(out=wpt[:, :, :], in_=wpD)

        # ---- PE prewarm: dummy matmuls run inside the DMA shadow, flipping
        # the HAM clock gate to 8/8 before real work arrives; sized to keep
        # the PE busy until the first real matmul's inputs have landed (a
        # re-throttle needs ~3.4us of PE idle, so a small tail gap is fine)
        warm = const.tile([128, 512], bf16, tag="warm")
        nc.vector.memset(warm[:, :], 0.0)
        for i in range(30):
            pw = ps.tile([128, 512], f32, tag="ps", name=f"warm{i}")
            nc.tensor.matmul(pw[:, :], lhsT=warm[:, 0:128], rhs=warm[:, :],
                             start=True, stop=True)

        # ---- v tile: [k%128, kblock, head, 64 dims + ones column]; 65-wide
        # weights keep the l-row riding in otherwise-unused M space
        vt_tile = vp.tile([128, NKB, 8, 65], bf16, tag="vt", name="vt")
        nc.vector.memset(vt_tile[:, :, :, 64:65], 1.0)

        def xT_cj(cj, lo, hi):
            q = lo // 512
            assert hi <= (q + 1) * 512
            return xT4[:, q, cj, lo - q * 512:hi - q * 512]

        vt = vt_tile

        def emit_vproj(i0, i1):
            for i in range(i0, i1):
                p_ = ps.tile([128, DH], f32, tag="ps", name=f"vps{i}")
                for cj in range(NCH):
                    nc.tensor.matmul(p_[:, :],
                                     lhsT=xT_cj(cj, i * 128, (i + 1) * 128),
                                     rhs=wvt[:, cj, :],
                                     start=(cj == 0), stop=(cj == NCH - 1))
                nc.vector.tensor_copy(
                    vt[:, i, :, 0:64],
                    p_[:, :].rearrange("p (h e) -> p h e", h=8))

        yn = [ynp.tile([128, T], bf16, tag=f"yn{hp}", name=f"yn{hp}")
              for hp in range(NPAIR)]
        qts, kts = {}, {}

        def emit_qkproj_part(hp, j):
            if hp not in qts:
                qts[hp] = qkp.tile([128, T], bf16, tag=f"qT{hp}",
                                   name=f"qT{hp}")
                kts[hp] = qkp.tile([128, T], bf16, tag=f"kT{hp}",
                                   name=f"kT{hp}")
            qt, kt = qts[hp], kts[hp]
            if True:
                pq = ps.tile([128, 512], f32, tag="ps", name=f"pq{hp}_{j}")
                for cj in range(NCH):
                    nc.tensor.matmul(
                        pq[:, :],
                        lhsT=wqt[:, hp, cj, :],
                        rhs=xT4[:, j, cj, :],
                        start=(cj == 0), stop=(cj == NCH - 1))
                nc.vector.tensor_scalar_add(qt[:, j * 512:(j + 1) * 512],
                                            pq[:, :], bq_sb[hp])
                pk = ps.tile([128, 512], f32, tag="ps", name=f"pk{hp}_{j}")
                for cj in range(NCH):
                    nc.tensor.matmul(
                        pk[:, :],
                        lhsT=wkt[:, hp, cj, :],
                        rhs=xT4[:, j, cj, :],
                        start=(cj == 0), stop=(cj == NCH - 1))
                nc.vector.tensor_scalar_add(kt[:, j * 512:(j + 1) * 512],
                                            pk[:, :], bk_sb[hp])

        def emit_attention(hp, Qi, chunk_filler=None):
                qt, kt = qts[hp], kts[hp]
                kmax = 4 * (Qi + 1)
                ya = yps.tile([65, 512], f32, tag="yps")
                yb = yps.tile([65, 512], f32, tag="yps")
                for ch in range(Qi + 1):
                    pts = []
                    for kb in range(4 * ch, 4 * ch + 4):
                        s = max(0, (kb - 4 * Qi) * 128)
                        sAB = ps2.tile([128, 2, 512], f32, tag="ps2")
                        nc.tensor.matmul(
                            sAB[:, 0, s:512],
                            lhsT=kt[0:64, kb * 128:(kb + 1) * 128],
                            rhs=qt[0:64, Qi * 512 + s:(Qi + 1) * 512],
                            start=True, stop=True)
                        nc.tensor.matmul(
                            sAB[:, 1, s:512],
                            lhsT=kt[64:128, kb * 128:(kb + 1) * 128],
                            rhs=qt[64:128, Qi * 512 + s:(Qi + 1) * 512],
                            start=True, stop=True)
                        pt_ = ptp.tile([128, 2, 512], bf16, tag="pt")
                        nc.scalar.activation(pt_[:, :, s:512],
                                             sAB[:, :, s:512],
                                             AF.Exp, scale=0.125)
                        if kb >= 4 * Qi:
                            nc.vector.tensor_mul(pt_[:, :, s:s + 128],
                                                 pt_[:, :, s:s + 128],
                                                 tri2[:, :, :])
                        pts.append((kb, s, pt_))
                    if chunk_filler is not None:
                        chunk_filler(ch)
                    for kb, s, pt_ in pts:
                        nc.tensor.matmul(ya[:, s:512],
                                         lhsT=vt[:, kb, 2 * hp, :],
                                         rhs=pt_[:, 0, s:512],
                                         start=(kb == 0), stop=(kb == kmax - 1))
                    for kb, s, pt_ in pts:
                        nc.tensor.matmul(yb[:, s:512],
                                         lhsT=vt[:, kb, 2 * hp + 1, :],
                                         rhs=pt_[:, 1, s:512],
                                         start=(kb == 0), stop=(kb == kmax - 1))
                # evacuate PSUM immediately, then a per-(pair,Qi) batched
                # reciprocal: l rows DMA-reshaped [1,512]->[128,4] so the
                # DVE reciprocal (~8 cyc/elem PER LANE) sees 8 elems/lane
                yevs = []
                for h, yy in ((0, ya), (1, yb)):
                    yev = yevp.tile([65, 512], f32, tag="yev")
                    nc.vector.tensor_copy(yev[:, :], yy[0:65, :])
                    yevs.append(yev)
                stg = stgp.tile([128, 8], f32, tag="stg")
                rstg = stgp.tile([128, 8], f32, tag="rstg")
                for h in (0, 1):
                    nc.sync.dma_start(out=stg[:, h * 4:(h + 1) * 4],
                                      in_=yevs[h][64:65, :])
                nc.vector.reciprocal(rstg[:, :], stg[:, :])
                for h in (0, 1):
                    rr = smallp.tile([1, 512], f32, tag="rr")
                    nc.sync.dma_start(out=rr[0:1, :],
                                      in_=rstg[:, h * 4:(h + 1) * 4])
                    bc = bcp.tile([64, 512], f32, tag="bc")
                    nc.gpsimd.partition_broadcast(bc[:, :], rr[0:1, :])
                    nc.vector.tensor_mul(
                        yn[hp][h * 64:(h + 1) * 64, Qi * 512:(Qi + 1) * 512],
                        yevs[h][0:64, :], bc[:, :])

        def emit_z(Qi):
            # runs in pair-3 territory where the qkv "ps" pool is otherwise
            # idle, so the z chains never touch the scores pipeline's banks
            for i in range(4 * Qi, 4 * Qi + 4):
                for j2 in range(2):
                    pz = ps.tile([128, 512], f32, tag="ps", name=f"pz{i}_{j2}")
                    for hp in range(NPAIR):
                        nc.tensor.matmul(
                            pz[:, :],
                            lhsT=yn[hp][:, i * 128:(i + 1) * 128],
                            rhs=wpt[:, hp, j2 * 512:(j2 + 1) * 512],
                            start=(hp == 0), stop=(hp == NPAIR - 1))
                    zs = zstp.tile([128, 512], mybir.dt.float16, tag="zst")
                    nc.vector.tensor_copy(zs[:, :], pz[:, :])
                    zeng = nc.sync if (i * 2 + j2) % 2 == 0 else nc.scalar
                    zeng.dma_start(
                        out=zD[i * 128:(i + 1) * 128,
                               j2 * 512:(j2 + 1) * 512],
                        in_=zs[:, :])

        # ---- schedule: pair 0's attention interleaves with the
        # v-projection so ScalarE's exp stream starts early; later pairs'
        # q/k projections are spread between the previous pair's attention
        # blocks (PE filler under the ACT-bound attention stretches);
        # pair 3 walks its query blocks in descending order with matching
        # z blocks right after, so the output projection chases pair 3.
        def vfill(rng):
            def f(ch):
                lo, hi = rng.get(ch, (None, None))
                if lo is not None:
                    emit_vproj(lo, hi)
            return f

        emit_qkproj_part(0, 0)
        emit_vproj(0, 4)
        emit_attention(0, 0)
        emit_qkproj_part(0, 1)
        emit_attention(0, 1, vfill({1: (4, 8)}))
        emit_qkproj_part(0, 2)
        emit_attention(0, 2, vfill({2: (8, 12)}))
        emit_qkproj_part(0, 3)

        def fill03(ch):
            if ch == 0:
                emit_qkproj_part(1, 0)
            elif ch == 1:
                emit_qkproj_part(1, 1)
            elif ch == 2:
                emit_vproj(12, 16)
            else:
                emit_qkproj_part(1, 2)

        emit_attention(0, 3, fill03)

        # q/k projections for the next pair are spread one part per chunk
        # across the previous stretches so the ACT-bound attention loops
        # always have PE filler
        QKFILL = {(1, 0): [(1, 3)], (1, 1): [(2, 0)],
                  (1, 2): [(2, 1)], (1, 3): [(2, 2), (2, 3)],
                  (2, 0): [(3, 0)], (2, 1): [(3, 1)],
                  (2, 2): [(3, 2)], (2, 3): [(3, 3)]}

        def qf(parts, Qi):
            def f(ch):
                if Qi == 0:
                    if ch == 0:
                        for p in parts:
                            emit_qkproj_part(*p)
                else:
                    i = ch - 1
                    if 0 <= i < len(parts):
                        emit_qkproj_part(*parts[i])
            return f

        for hp in (1, 2):
            for Qi in range(NQB):
                emit_attention(hp, Qi, qf(QKFILL[(hp, Qi)], Qi))

        # pair 3 walks its query blocks in descending order; each z block is
        # emitted as filler inside the NEXT attention stretch so its wait on
        # the normalize chain overlaps that stretch's ACT-bound bubbles
        def zf(Qi_z, trig):
            def f(ch):
                if ch == trig:
                    emit_z(Qi_z)
            return f

        emit_attention(3, 3)
        emit_attention(3, 2, zf(3, 1))
        emit_attention(3, 1, zf(2, 1))
        emit_attention(3, 0, zf(1, 0))
        emit_z(0)

    nc.compile()
    return nc


def get_nc():
    global _CACHED_NC
    if _CACHED_NC is None:
        _CACHED_NC = _build_nc()
    return _CACHED_NC


def make_in_map(core, x, Wq, bq, Wk, bk, Wv, Wp):
    """Host-side shard/layout prep for one core (pure numpy, no FLOP-bearing
    compute: transposes, slicing, dtype casts). All device tensors are
    pre-arranged so each SBUF partition's slab is contiguous in DRAM."""
    b, r = core // 2, core % 2
    hsl = slice(r * DH, (r + 1) * DH)
    xt = np.ascontiguousarray(x[b].T).astype(BF)            # [C, T]
    xT4 = np.ascontiguousarray(
        xt.reshape(NCH, 128, 4, 512).transpose(2, 1, 0, 3))  # [4,128,8,512]
    wq = np.ascontiguousarray(
        Wq[hsl, :].T.reshape(NCH, 128, NPAIR, 128)
        .transpose(1, 2, 0, 3)).astype(BF)                   # [128,4,8,128]
    wk = np.ascontiguousarray(
        Wk[hsl, :].T.reshape(NCH, 128, NPAIR, 128)
        .transpose(1, 2, 0, 3)).astype(BF)
    wv = np.ascontiguousarray(
        Wv[hsl, :].T.reshape(NCH, 128, DH)
        .transpose(1, 0, 2)).astype(BF)                      # [128,8,512]
    wp = np.ascontiguousarray(
        Wp[:, hsl].T.reshape(NPAIR, 128, C)
        .transpose(1, 0, 2)).astype(BF)                      # [128,4,1024]
    bqk = np.ascontiguousarray(
        np.stack([bq[hsl].reshape(NPAIR, 128, 1),
                  bk[hsl].reshape(NPAIR, 128, 1)])
        .transpose(2, 0, 1, 3)).astype(np.float32)           # [128,2,4,1]
    tri1 = np.triu(np.ones((128, 128), np.float32)).astype(BF)
    tri = np.ascontiguousarray(
        np.broadcast_to(tri1[:, None, :], (128, 2, 128)))
    return {
        "xT4": xT4,
        "wq": wq,
        "wk": wk,
        "wv": wv,
        "wp": wp,
        "bqk": bqk,
        "tri": tri,
    }


def kernel(x, Wq, bq, Wk, bk, Wv, bv, Wp):
    global LAST_RESULTS
    x = np.asarray(x, np.float32)
    Wq, bq = np.asarray(Wq, np.float32), np.asarray(bq, np.float32)
    Wk, bk = np.asarray(Wk, np.float32), np.asarray(bk, np.float32)
    Wv, bv = np.asarray(Wv, np.float32), np.asarray(bv, np.float32)
    Wp = np.asarray(Wp, np.float32)

    nc = get_nc()
    in_maps = [make_in_map(c, x, Wq, bq, Wk, bk, Wv, Wp)
               for c in range(NCORES)]
    res = None
    for attempt in range(3):
        try:
            res = run_bass_kernel_spmd(nc, in_maps,
                                       core_ids=list(range(NCORES)))
            break
        except Exception:
            if attempt == 2:
                raise
            import time
            time.sleep(5)
    LAST_RESULTS = res

    # unshard: sum the two head-half partials per batch; add folded V-bias
    # term (y gets +bv per token; through the output projection that is the
    # constant vector Wp @ bv added to every token)
    zbias = (Wp @ bv).astype(np.float32)
    out = np.empty((B, T, C), np.float32)
    for b in range(B):
        za = np.asarray(res.results[2 * b]["z"], np.float32)
        zb = np.asarray(res.results[2 * b + 1]["z"], np.float32)
        out[b] = za + zb + zbias[None, :]
    return out


# revision 16
# speedup vs baseline: 1.1772x; 1.1772x over previous
"""Causal multi-head attention (B=4, T=2048, C=1024, H=16) on 8 TRN2 NeuronCores.

Sharding: core c handles batch b=c//2 and head-half r=c%2 (8 of 16 heads).
Every core runs an IDENTICAL graph (full causal attention for its 8 heads over
all T tokens) -> pure SPMD, no collectives. The output projection is
row-parallel over the head-halves; the host sums the two partial z's per batch
(the unshard step) and adds the bias-fold vector Wp@bv.

Device layout choices:
  - everything enters as bf16 (host pre-casts); matmuls accumulate fp32 in PSUM
  - all DRAM inputs are PRE-ARRANGED on the host to match their SBUF
    destination exactly (partition-major slabs) -> every load DMA is one
    contiguous descriptor per partition, so inputs land at line rate
  - a short chain of dummy matmuls on a memset tile runs during the input
    DMA shadow, warming the PE HAM clock gate before the first real matmul
  - qT/kT stored [d, t] with head pairs stacked 64+64 on partitions ->
    K=64 row-tiled matmul pairs use both halves of the PE array concurrently
  - scores computed transposed S^T=[k, q]; exp on ScalarE (scale=1/8 fused);
    causal handled by N-trimming each matmul + one 128x128 triangle mask mul
  - softmax denominator l = sum_k exp computed for free by an all-ones column
    appended to v (65-wide weights, fp32 PSUM accumulation); y^T = v_aug^T@P^T
  - 1/l via ScalarE Ln then Exp(scale=-1) directly on the [1, 2, 512] l-rows
    (both functions live in one ACT table set), then one gpsimd
    partition_broadcast; normalize is a bf16 DVE multiply
"""

import os
from contextlib import ExitStack

import numpy as np
import ml_dtypes

import concourse.tile as tile
from concourse import bacc, mybir


def _ensure_axon_hooks():
    """bass_utils' axon trace path does a hard import of antenv.axon_hooks,
    which this image's antenv lacks. Provide the module (with a real NTFF
    hook when the axon .so supports it) so trace=True / BASS_TRACE=1 works;
    harmless when tracing is off."""
    try:
        import antenv.axon_hooks  # noqa: F401
        return
    except ImportError:
        pass
    import sys
    import types
    try:
        import antenv
    except ImportError:
        return
    m = types.ModuleType("antenv.axon_hooks")
    m._hook = None

    def set_axon_ntff_profile_hook(h):
        m._hook = h

    def get_axon_ntff_profile_hook():
        return m._hook

    m.set_axon_ntff_profile_hook = set_axon_ntff_profile_hook
    m.get_axon_ntff_profile_hook = get_axon_ntff_profile_hook
    sys.modules["antenv.axon_hooks"] = m
    antenv.axon_hooks = m


_ensure_axon_hooks()

from concourse.bass_utils import run_bass_kernel_spmd  # noqa: E402

BF = ml_dtypes.bfloat16
B, T, C, H, HD = 4, 2048, 1024, 16, 64
NCORES = 8
DH = C // 2        # 512 d-dims per core (8 heads)
NPAIR = 4          # head pairs per core
NQB = T // 512     # 4 query blocks of 512
NKB = T // 128     # 16 key/token blocks of 128
NCH = C // 128     # 8 contraction chunks
f32 = mybir.dt.float32
bf16 = mybir.dt.bfloat16

_CACHED_NC = None
LAST_RESULTS = None  # BassKernelResults of the most recent run


def _build_nc():
    nc = bacc.Bacc("TRN2", target_bir_lowering=False, debug=False,
                   num_devices=NCORES)
    AF = mybir.ActivationFunctionType

    # DRAM inputs pre-arranged host-side: leading dim = SBUF partition, and
    # each partition's slab is contiguous -> 1 DMA descriptor per partition.
    xT4D = nc.dram_tensor("xT4", [4, 128, NCH, 512], bf16,
                          kind="ExternalInput").ap()
    wqD = nc.dram_tensor("wq", [128, NPAIR, NCH, 128], bf16,
                         kind="ExternalInput").ap()
    wkD = nc.dram_tensor("wk", [128, NPAIR, NCH, 128], bf16,
                         kind="ExternalInput").ap()
    wvD = nc.dram_tensor("wv", [128, NCH, DH], bf16,
                         kind="ExternalInput").ap()
    wpD = nc.dram_tensor("wp", [128, NPAIR, C], bf16,
                         kind="ExternalInput").ap()
    bqkD = nc.dram_tensor("bqk", [128, 2, NPAIR, 1], f32,
                          kind="ExternalInput").ap()
    triD = nc.dram_tensor("tri", [128, 2, 128], bf16,
                          kind="ExternalInput").ap()
    zD = nc.dram_tensor("z", [T, C], mybir.dt.float16,
                        kind="ExternalOutput").ap()

    with tile.TileContext(nc) as tc, ExitStack() as ctx:
        const = ctx.enter_context(tc.tile_pool(name="const", bufs=1))
        qkp = ctx.enter_context(tc.tile_pool(name="qk", bufs=1))
        vp = ctx.enter_context(tc.tile_pool(name="vp", bufs=1))
        ynp = ctx.enter_context(tc.tile_pool(name="yn", bufs=1))
        ptp = ctx.enter_context(tc.tile_pool(name="pt", bufs=7))
        smallp = ctx.enter_context(tc.tile_pool(name="small", bufs=2))
        bcp = ctx.enter_context(tc.tile_pool(name="bc", bufs=2))
        zstp = ctx.enter_context(tc.tile_pool(name="zst", bufs=3))
        yevp = ctx.enter_context(tc.tile_pool(name="yev", bufs=6))
        stgp = ctx.enter_context(tc.tile_pool(name="stg", bufs=2))
        ps = ctx.enter_context(tc.tile_pool(name="ps", bufs=2, space="PSUM"))
        ps2 = ctx.enter_context(tc.tile_pool(name="ps2", bufs=2, space="PSUM"))
        yps = ctx.enter_context(tc.tile_pool(name="yps", bufs=2, space="PSUM"))

        # ---- input DMAs first so every queue starts pulling immediately
        tri2 = const.tile([128, 2, 128], bf16, tag="tri2")
        nc.gpsimd.dma_start(out=tri2[:, :, :], in_=triD)
        bqk = const.tile([128, 2, NPAIR, 1], f32, tag="bqk")
        nc.gpsimd.dma_start(out=bqk[:, :, :, :], in_=bqkD)
        bq_sb = [bqk[:, 0, hp, :] for hp in range(NPAIR)]
        bk_sb = [bqk[:, 1, hp, :] for hp in range(NPAIR)]

        xT4 = const.tile([128, 4, NCH, 512], bf16, tag="xT4", name="xT4")
        wqt = const.tile([128, NPAIR, NCH, 128], bf16, tag="wqt", name="wqt")
        wkt = const.tile([128, NPAIR, NCH, 128], bf16, tag="wkt", name="wkt")
        wvt = const.tile([128, NCH, DH], bf16, tag="wvt", name="wvt")
        wpt = const.tile([128, NPAIR, C], bf16, tag="wpt", name="wpt")

        # wave order tuned to the per-queue DMA rates (~100-140GB/s on the
        # sync/scalar HW queues, ~45GB/s on gpsimd's SWDGE queue) and to when
        # each consumer fires: pair-0 j=0 projections first, v weights early
        # on scalar so the v-projection fills the pre-attention bubble, the
        # remaining token quarters just-in-time behind them
        nc.sync.dma_start(out=wqt[:, 0:1, :, :], in_=wqD[:, 0:1, :, :])
        nc.scalar.dma_start(out=wkt[:, 0:1, :, :], in_=wkD[:, 0:1, :, :])
        for c2 in range(4):
            nc.sync.dma_start(out=xT4[:, 0, 2 * c2:2 * c2 + 2, :],
                              in_=xT4D[0, :, 2 * c2:2 * c2 + 2, :])
        nc.scalar.dma_start(out=wvt[:, :, :], in_=wvD)
        for hh in range(1, NPAIR):
            nc.sync.dma_start(out=wqt[:, hh:hh + 1, :, :],
                              in_=wqD[:, hh:hh + 1, :, :])
            nc.scalar.dma_start(out=wkt[:, hh:hh + 1, :, :],
                                in_=wkD[:, hh:hh + 1, :, :])
        nc.sync.dma_start(out=xT4[:, 1, :, :], in_=xT4D[1])
        nc.scalar.dma_start(out=xT4[:, 2, :, :], in_=xT4D[2])
        nc.sync.dma_start(out=xT4[:, 3, :, :], in_=xT4D[3])
        nc.gpsimd.dma_start(out=wpt[:, :, :], in_=wpD)

        # ---- PE prewarm: dummy matmuls run inside the DMA shadow, flipping
        # the HAM clock gate to 8/8 before real work arrives; sized to keep
        # the PE busy until the first real matmul's inputs have landed (a
        # re-throttle needs ~3.4us of PE idle, so a small tail gap is fine)
        warm = const.tile([128, 512], bf16, tag="warm")
        nc.vector.memset(warm[:, :], 0.0)
        for i in range(30):
            pw = ps.tile([128, 512], f32, tag="ps", name=f"warm{i}")
            nc.tensor.matmul(pw[:, :], lhsT=warm[:, 0:128], rhs=warm[:, :],
                             start=True, stop=True)

        # ---- v tile: [k%128, kblock, head, 64 dims + ones column]; 65-wide
        # weights keep the l-row riding in otherwise-unused M space
        vt_tile = vp.tile([128, NKB, 8, 65], bf16, tag="vt", name="vt")
        nc.vector.memset(vt_tile[:, :, :, 64:65], 1.0)

        def xT_cj(cj, lo, hi):
            q = lo // 512
            assert hi <= (q + 1) * 512
            return xT4[:, q, cj, lo - q * 512:hi - q * 512]

        vt = vt_tile

        def emit_vproj(i0, i1):
            for i in range(i0, i1):
                p_ = ps.tile([128, DH], f32, tag="ps", name=f"vps{i}")
                for cj in range(NCH):
                    nc.tensor.matmul(p_[:, :],
                                     lhsT=xT_cj(cj, i * 128, (i + 1) * 128),
                                     rhs=wvt[:, cj, :],
                                     start=(cj == 0), stop=(cj == NCH - 1))
                nc.vector.tensor_copy(
                    vt[:, i, :, 0:64],
                    p_[:, :].rearrange("p (h e) -> p h e", h=8))

        yn = [ynp.tile([128, T], bf16, tag=f"yn{hp}", name=f"yn{hp}")
              for hp in range(NPAIR)]
        qts, kts = {}, {}

        def emit_qkproj_part(hp, j):
            if hp not in qts:
                qts[hp] = qkp.tile([128, T], bf16, tag=f"qT{hp}",
                                   name=f"qT{hp}")
                kts[hp] = qkp.tile([128, T], bf16, tag=f"kT{hp}",
                                   name=f"kT{hp}")
            qt, kt = qts[hp], kts[hp]
            if True:
                pq = ps.tile([128, 512], f32, tag="ps", name=f"pq{hp}_{j}")
                for cj in range(NCH):
                    nc.tensor.matmul(
                        pq[:, :],
                        lhsT=wqt[:, hp, cj, :],
                        rhs=xT4[:, j, cj, :],
                        start=(cj == 0), stop=(cj == NCH - 1))
                nc.vector.tensor_scalar_add(qt[:, j * 512:(j + 1) * 512],
                                            pq[:, :], bq_sb[hp])
                pk = ps.tile([128, 512], f32, tag="ps", name=f"pk{hp}_{j}")
                for cj in range(NCH):
                    nc.tensor.matmul(
                        pk[:, :],
                        lhsT=wkt[:, hp, cj, :],
                        rhs=xT4[:, j, cj, :],
                        start=(cj == 0), stop=(cj == NCH - 1))
                nc.vector.tensor_scalar_add(kt[:, j * 512:(j + 1) * 512],
                                            pk[:, :], bk_sb[hp])

        def emit_attention(hp, Qi, chunk_filler=None):
                qt, kt = qts[hp], kts[hp]
                kmax = 4 * (Qi + 1)
                ya = yps.tile([65, 512], f32, tag="yps")
                yb = yps.tile([65, 512], f32, tag="yps")
                for ch in range(Qi + 1):
                    pts = []
                    for kb in range(4 * ch, 4 * ch + 4):
                        s = max(0, (kb - 4 * Qi) * 128)
                        sAB = ps2.tile([128, 2, 512], f32, tag="ps2")
                        nc.tensor.matmul(
                            sAB[:, 0, s:512],
                            lhsT=kt[0:64, kb * 128:(kb + 1) * 128],
                            rhs=qt[0:64, Qi * 512 + s:(Qi + 1) * 512],
                            start=True, stop=True)
                        nc.tensor.matmul(
                            sAB[:, 1, s:512],
                            lhsT=kt[64:128, kb * 128:(kb + 1) * 128],
                            rhs=qt[64:128, Qi * 512 + s:(Qi + 1) * 512],
                            start=True, stop=True)
                        pt_ = ptp.tile([128, 2, 512], bf16, tag="pt")
                        nc.scalar.activation(pt_[:, :, s:512],
                                             sAB[:, :, s:512],
                                             AF.Exp, scale=0.125)
                        if kb >= 4 * Qi:
                            nc.vector.tensor_mul(pt_[:, :, s:s + 128],
                                                 pt_[:, :, s:s + 128],
                                                 tri2[:, :, :])
                        pts.append((kb, s, pt_))
                    if chunk_filler is not None:
                        chunk_filler(ch)
                    for kb, s, pt_ in pts:
                        nc.tensor.matmul(ya[:, s:512],
                                         lhsT=vt[:, kb, 2 * hp, :],
                                         rhs=pt_[:, 0, s:512],
                                         start=(kb == 0), stop=(kb == kmax - 1))
                    for kb, s, pt_ in pts:
                        nc.tensor.matmul(yb[:, s:512],
                                         lhsT=vt[:, kb, 2 * hp + 1, :],
                                         rhs=pt_[:, 1, s:512],
                                         start=(kb == 0), stop=(kb == kmax - 1))
                # evacuate PSUM immediately, then a per-(pair,Qi) batched
                # reciprocal: l rows DMA-reshaped [1,512]->[128,4] so the
                # DVE reciprocal (~8 cyc/elem PER LANE) sees 8 elems/lane
                yevs = []
                for h, yy in ((0, ya), (1, yb)):
                    yev = yevp.tile([65, 512], f32, tag="yev")
                    nc.vector.tensor_copy(yev[:, :], yy[0:65, :])
                    yevs.append(yev)
                stg = stgp.tile([128, 8], f32, tag="stg")
                rstg = stgp.tile([128, 8], f32, tag="rstg")
                for h in (0, 1):
                    nc.sync.dma_start(out=stg[:, h * 4:(h + 1) * 4],
                                      in_=yevs[h][64:65, :])
                nc.vector.reciprocal(rstg[:, :], stg[:, :])
                for h in (0, 1):
                    rr = smallp.tile([1, 512], f32, tag="rr")
                    nc.sync.dma_start(out=rr[0:1, :],
                                      in_=rstg[:, h * 4:(h + 1) * 4])
                    bc = bcp.tile([64, 512], f32, tag="bc")
                    nc.gpsimd.partition_broadcast(bc[:, :], rr[0:1, :])
                    nc.vector.tensor_mul(
                        yn[hp][h * 64:(h + 1) * 64, Qi * 512:(Qi + 1) * 512],
                        yevs[h][0:64, :], bc[:, :])

        def emit_z(Qi):
            # runs in pair-3 territory where the qkv "ps" pool is otherwise
            # idle, so the z chains never touch the scores pipeline's banks
            for i in range(4 * Qi, 4 * Qi + 4):
                for j2 in range(2):
                    pz = ps.tile([128, 512], f32, tag="ps", name=f"pz{i}_{j2}")
                    for hp in range(NPAIR):
                        nc.tensor.matmul(
                            pz[:, :],
                            lhsT=yn[hp][:, i * 128:(i + 1) * 128],
                            rhs=wpt[:, hp, j2 * 512:(j2 + 1) * 512],
                            start=(hp == 0), stop=(hp == NPAIR - 1))
                    zs = zstp.tile([128, 512], mybir.dt.float16, tag="zst")
                    nc.vector.tensor_copy(zs[:, :], pz[:, :])
                    zeng = nc.sync if (i * 2 + j2) % 2 == 0 else nc.scalar
                    zeng.dma_start(
                        out=zD[i * 128:(i + 1) * 128,
                               j2 * 512:(j2 + 1) * 512],
                        in_=zs[:, :])

        # ---- schedule: pair 0's attention interleaves with the
        # v-projection so ScalarE's exp stream starts early; later pairs'
        # q/k projections are spread between the previous pair's attention
        # blocks (PE filler under the ACT-bound attention stretches);
        # pair 3 walks its query blocks in descending order with matching
        # z blocks right after, so the output projection chases pair 3.
        def vfill(rng):
            def f(ch):
                lo, hi = rng.get(ch, (None, None))
                if lo is not None:
                    emit_vproj(lo, hi)
            return f

        emit_qkproj_part(0, 0)
        emit_vproj(0, 4)
        emit_attention(0, 0)
        emit_qkproj_part(0, 1)
        emit_attention(0, 1, vfill({1: (4, 8)}))
        emit_qkproj_part(0, 2)
        emit_attention(0, 2, vfill({2: (8, 12)}))
        emit_qkproj_part(0, 3)

        def pfill(hp_next):
            def f(ch):
                if 1 <= ch <= 3:
                    emit_qkproj_part(hp_next, ch)
            return f

        def fill03(ch):
            if ch < 3:
                emit_qkproj_part(1, ch)
            else:
                emit_vproj(12, 16)
                emit_qkproj_part(1, 3)

        emit_attention(0, 3, fill03)
        for hp in (1, 2):
            for Qi in range(NQB - 1):
                emit_attention(hp, Qi)
            emit_qkproj_part(hp + 1, 0)
            emit_attention(hp, NQB - 1, pfill(hp + 1))

        # pair 3 walks its query blocks in descending order; each z block is
        # emitted after the NEXT attention stretch so its wait on the
        # normalize chain overlaps that stretch instead of stalling the PE
        emit_attention(3, 3)
        emit_attention(3, 2)
        emit_z(3)
        emit_attention(3, 1)
        emit_z(2)
        emit_attention(3, 0)
        emit_z(1)
        emit_z(0)

    nc.compile()
    return nc


def get_nc():
    global _CACHED_NC
    if _CACHED_NC is None:
        _CACHED_NC = _build_nc()
    return _CACHED_NC


def make_in_map(core, x, Wq, bq, Wk, bk, Wv, Wp):
    """Host-side shard/layout prep for one core (pure numpy, no FLOP-bearing
    compute: transposes, slicing, dtype casts). All device tensors are
    pre-arranged so each SBUF partition's slab is contiguous in DRAM."""
    b, r = core // 2, core % 2
    hsl = slice(r * DH, (r + 1) * DH)
    xt = np.ascontiguousarray(x[b].T).astype(BF)            # [C, T]
    xT4 = np.ascontiguousarray(
        xt.reshape(NCH, 128, 4, 512).transpose(2, 1, 0, 3))  # [4,128,8,512]
    wq = np.ascontiguousarray(
        Wq[hsl, :].T.reshape(NCH, 128, NPAIR, 128)
        .transpose(1, 2, 0, 3)).astype(BF)                   # [128,4,8,128]
    wk = np.ascontiguousarray(
        Wk[hsl, :].T.reshape(NCH, 128, NPAIR, 128)
        .transpose(1, 2, 0, 3)).astype(BF)
    wv = np.ascontiguousarray(
        Wv[hsl, :].T.reshape(NCH, 128, DH)
        .transpose(1, 0, 2)).astype(BF)                      # [128,8,512]
    wp = np.ascontiguousarray(
        Wp[:, hsl].T.reshape(NPAIR, 128, C)
        .transpose(1, 0, 2)).astype(BF)                      # [128,4,1024]
    bqk = np.ascontiguousarray(
        np.stack([bq[hsl].reshape(NPAIR, 128, 1),
                  bk[hsl].reshape(NPAIR, 128, 1)])
        .transpose(2, 0, 1, 3)).astype(np.float32)           # [128,2,4,1]
    tri1 = np.triu(np.ones((128, 128), np.float32)).astype(BF)
    tri = np.ascontiguousarray(
        np.broadcast_to(tri1[:, None, :], (128, 2, 128)))
    return {
        "xT4": xT4,
        "wq": wq,
        "wk": wk,
        "wv": wv,
        "wp": wp,
        "bqk": bqk,
        "tri": tri,
    }


def kernel(x, Wq, bq, Wk, bk, Wv, bv, Wp):
    global LAST_RESULTS
    x = np.asarray(x, np.float32)
    Wq, bq = np.asarray(Wq, np.float32), np.asarray(bq, np.float32)
    Wk, bk = np.asarray(Wk, np.float32), np.asarray(bk, np.float32)
    Wv, bv = np.asarray(Wv, np.float32), np.asarray(bv, np.float32)
    Wp = np.asarray(Wp, np.float32)

    nc = get_nc()
    in_maps = [make_in_map(c, x, Wq, bq, Wk, bk, Wv, Wp)
               for c in range(NCORES)]
    res = None
    for attempt in range(3):
        try:
            res = run_bass_kernel_spmd(nc, in_maps,
                                       core_ids=list(range(NCORES)))
            break
        except Exception:
            if attempt == 2:
                raise
            import time
            time.sleep(5)
    LAST_RESULTS = res

    # unshard: sum the two head-half partials per batch; add folded V-bias
    # term (y gets +bv per token; through the output projection that is the
    # constant vector Wp @ bv added to every token)
    zbias = (Wp @ bv).astype(np.float32)
    out = np.empty((B, T, C), np.float32)
    for b in range(B):
        za = np.asarray(res.results[2 * b]["z"], np.float32)
        zb = np.asarray(res.results[2 * b + 1]["z"], np.float32)
        out[b] = za + zb + zbias[None, :]
    return out
